# revision 22
# baseline (speedup 1.0000x reference)
"""Distributed GQA attention block (RMSNorm-QK + RoPE + causal attention + Wo
+ final RMSNorm) for one TRN2 chip (8 NeuronCores).

Sharding: tensor-parallel over heads. Core i computes q-heads {2i, 2i+1} and
kv-head i//2. Everything on-device is computed transposed ([dim, seq]) so the
hidden/contraction axis lands on SBUF partitions with zero on-device
transposes of X. An AllToAll redistributes the attention output from
head-sharded to sequence-sharded; each core then runs the output projection
and final RMSNorm for its own 256 tokens.

v3 structure: projection/norm/rope and head-0 attention are software-
pipelined per 512-token chunk; head-1 attention runs after the first
AllToAll is issued so the collective hides under it; the output projection
starts on the even head-blocks as soon as the first AllToAll lands. All
rsqrt/reciprocal chains run as Ln+Exp on the scalar engine (single
activation-table set, no serialized [1,N] DVE reciprocals).

Numerics: bf16 matmuls with f32 PSUM accumulation; softmax is computed
without max-subtraction (logits are O(1) here: RMS-normed q/k and 1/sqrt(D)
scaling), with the causal mask applied multiplicatively after exp.
"""

import sys

sys.path.insert(0, "/opt/trn_rl_repo")

import numpy as np
import ml_dtypes

BF16 = ml_dtypes.bfloat16

S = 2048  # sequence length
H = 2048  # hidden
D = 128  # head dim
NH = 16  # query heads
NKV = 4  # kv heads
NC = 8  # cores
HL = NH // NC  # q heads per core = 2
SC = S // NC  # seq per core (output shard) = 256
CH = 4  # seq chunks
CW = 512  # chunk width
KT = H // 128  # contraction tiles = 16
EPS = 1e-6
SQRT_D = float(np.sqrt(D))

_cache: dict = {}


def _patch_act_tables():
    """Force all activations into the natural_log_exp_and_others table set.

    bacc's insert_act_table_loads picks the FIRST act_info.json set
    containing each function, so alternating Ln/Exp thrashes between
    `natural_log` and `exp_and_others` (~1.3us per ACT_TABLE_LOAD, dozens
    per kernel). Blanking every other set (order/indices preserved, so the
    emitted act_func_set_id still matches act_info.json) makes Ln, Exp and
    Copy all resolve to the one combined set -> a single table load.
    """
    import concourse.bacc as bacc_mod
    import concourse.hw_specs as hw

    orig = hw.get_activation_tables
    if getattr(bacc_mod.get_activation_tables, "_combined_set_only", False):
        return
    KEEP = "natural_log_exp_and_others"

    def patched(arch):
        t = dict(orig(arch))
        return {k: (v if k == KEEP else set()) for k, v in t.items()}

    patched._combined_set_only = True
    bacc_mod.get_activation_tables = patched


def _build_nc(reps: int = 1):
    import concourse.bass as bass
    import concourse.tile as tile
    from concourse import bacc, mybir

    _patch_act_tables()

    f32 = mybir.dt.float32
    bf16 = mybir.dt.bfloat16
    AF = mybir.ActivationFunctionType

    nc = bacc.Bacc("TRN2", target_bir_lowering=False, debug=False, num_devices=NC)

    # ---- kernel I/O (per-core shards; replicated where noted) ----
    xt_d = nc.dram_tensor("xt", [H, S], bf16, kind="ExternalInput").ap()  # X^T
    wq_d = nc.dram_tensor("wq", [128, KT, HL * D], bf16, kind="ExternalInput").ap()
    wk_d = nc.dram_tensor("wk", [128, KT, D], bf16, kind="ExternalInput").ap()
    wv_d = nc.dram_tensor("wv", [128, KT, D], bf16, kind="ExternalInput").ap()
    wo_d = nc.dram_tensor("wo", [H, H], bf16, kind="ExternalInput").ap()
    cq_d = nc.dram_tensor("cq", [D, S], bf16, kind="ExternalInput").ap()
    sq_d = nc.dram_tensor("sq", [D, S], bf16, kind="ExternalInput").ap()
    ck_d = nc.dram_tensor("ck", [D, S], bf16, kind="ExternalInput").ap()
    sk_d = nc.dram_tensor("sk", [D, S], bf16, kind="ExternalInput").ap()
    msk_d = nc.dram_tensor("msk", [D, 4, CW], bf16, kind="ExternalInput").ap()
    rot_d = nc.dram_tensor("rot", [D, D], bf16, kind="ExternalInput").ap()
    ls_d = nc.dram_tensor("ls", [1, H], f32, kind="ExternalInput").ap()
    out_d = nc.dram_tensor("out", [SC, H], f32, kind="ExternalOutput").ap()

    with tile.TileContext(nc) as tc:
        with (
            tc.tile_pool(name="singles", bufs=1) as singles,
            tc.tile_pool(name="work", bufs=3) as work,
            tc.tile_pool(name="xtp", bufs=18) as xtp,
            tc.tile_pool(name="wop", bufs=8) as wop,
            tc.tile_pool(name="small", bufs=2) as small,
            tc.tile_pool(name="pa", bufs=3, space="PSUM") as pa,
            tc.tile_pool(name="pc", bufs=4, space="PSUM") as pc,
            tc.tile_pool(name="psm", bufs=1, space="PSUM") as psm,
            tc.tile_pool(name="dram", bufs=1, space="DRAM") as dram,
        ):
            # ---------- resident SBUF tensors ----------
            # q/k/v weights first: the first projection matmuls need them
            wq_sb = singles.tile([128, KT, HL * D], bf16)
            wk_sb = singles.tile([128, KT, D], bf16)
            wv_sb = singles.tile([128, KT, D], bf16)
            for g in range(4):
                gs = slice(4 * g, 4 * g + 4)
                nc.sync.dma_start(wq_sb[:, gs, :], wq_d[:, gs, :])
                nc.sync.dma_start(wk_sb[:, gs, :], wk_d[:, gs, :])
                nc.sync.dma_start(wv_sb[:, gs, :], wv_d[:, gs, :])
            rot_sb = singles.tile([128, 128], bf16)  # R^T: rotate-half as matmul
            nc.sync.dma_start(rot_sb, rot_d)
            cq_sb = singles.tile([128, S], bf16)
            sq_sb = singles.tile([128, S], bf16)
            ck_sb = singles.tile([128, S], bf16)
            sk_sb = singles.tile([128, S], bf16)
            for hh in range(2):
                hs = slice(hh * 1024, (hh + 1) * 1024)
                nc.sync.dma_start(cq_sb[:, hs], cq_d[:, hs])
                nc.sync.dma_start(sq_sb[:, hs], sq_d[:, hs])
                nc.sync.dma_start(ck_sb[:, hs], ck_d[:, hs])
                nc.sync.dma_start(sk_sb[:, hs], sk_d[:, hs])
            msk_sb = singles.tile([128, 4, CW], bf16)
            nc.sync.dma_start(msk_sb, msk_d)
            ls_sb = singles.tile([128, H], f32)  # last_norm_scale bcast over parts
            nc.sync.dma_start(ls_sb, ls_d.to_broadcast([128, H]))

            ones_col_bf = singles.tile([128, 1], bf16)  # lhsT for partition sums
            nc.vector.memset(ones_col_bf, 1.0)
            eps_row = singles.tile([1, 1], f32)  # D*eps for k sumsq
            nc.vector.memset(eps_row, D * EPS)
            eps_one = singles.tile([1, 1], f32)  # eps for q mean-sumsq
            nc.vector.memset(eps_one, EPS)
            eps_col = singles.tile([128, 1], f32)  # eps for final norm
            nc.vector.memset(eps_col, EPS)

            qt_sb = singles.tile([128, HL, S], bf16)  # roped Q^T per local head
            kt_sb = singles.tile([128, S], bf16)  # roped K^T (rs_k deferred)
            vt_sb = singles.tile([128, S], bf16)  # V^T (pre-transpose)
            v_sb = singles.tile([128, KT, D], bf16)  # V tiles [j-part, jt, d]
            rsk_cols = [
                singles.tile([128, 4], f32, name=f"rskc{c}") for c in range(CH)
            ]  # partition layout, per source chunk
            ot_sb = singles.tile([128, HL, S], bf16)  # normalized attn out^T
            att_sb = singles.tile([128, KT, HL, 128], bf16)  # att^T post-A2A
            y_sb = singles.tile([128, 2, H], bf16)  # Y rows (2 x 128 tokens)
            pt_sb = singles.tile([128, 2, 4], f32)  # sumsq partials per col group
            a2a_in = [
                dram.tile([S // 2, SC], bf16, name=f"a2ai{j}") for j in range(HL)
            ]
            a2a_out = [
                dram.tile([S // 2, SC], bf16, name=f"a2ao{j}") for j in range(HL)
            ]
            rsk_dram = dram.tile([1, S], f32)

            for _rep in range(reps):
                # ---------- per-chunk: QKV projection, RMSNorm, RoPE ----------
                def norm_rope(src_ps, c, cos_sb, sin_sb, dst, is_q):
                    """src_ps: [128,CW] f32 PSUM (pre-norm, transposed)."""
                    csl = slice(c * CW, (c + 1) * CW)
                    raw = work.tile([128, CW], bf16, tag="raw", bufs=3)
                    nc.vector.tensor_copy(raw, src_ps)  # frees the PSUM bank
                    sq2 = work.tile([128, CW], bf16, tag="sq2")
                    nc.vector.tensor_mul(sq2, raw, raw)
                    ssq = psm.tile([1, CW], f32, tag="ssq")
                    nc.tensor.matmul(ssq, ones_col_bf, sq2, start=True, stop=True)
                    # rope on raw runs in parallel with the rs chain
                    rot_ps = pc.tile([128, CW], f32, tag="big", name="rotps")
                    nc.tensor.matmul(rot_ps, rot_sb, raw, start=True, stop=True)
                    t1 = work.tile([128, CW], bf16, tag="t1")
                    nc.vector.tensor_mul(t1, raw, cos_sb[:, csl])
                    t2 = work.tile([128, CW], bf16, tag="t2")
                    nc.vector.tensor_mul(t2, rot_ps, sin_sb[:, csl])
                    if is_q:
                        # rs = (ssq/D + eps)^-1/2 via Ln+Exp (one act-table set)
                        lt = small.tile([1, CW], f32, tag="lt")
                        nc.scalar.activation(lt, ssq, AF.Ln, bias=eps_one, scale=1.0 / D)
                        rs_bf = small.tile([1, CW], bf16, tag="rs_bf")
                        nc.scalar.activation(rs_bf, lt, AF.Exp, scale=-0.5)
                        rs_dr = dram.tile([1, CW], bf16, tag="rs_dr", bufs=3, name="rs_dr")
                        nc.sync.dma_start(rs_dr, rs_bf)
                        rsb = work.tile([128, CW], bf16, tag="rsb", bufs=3)
                        nc.sync.dma_start(rsb, rs_dr[:, :].to_broadcast([128, CW]))
                        rp = work.tile([128, CW], bf16, tag="rp")
                        nc.vector.tensor_add(rp, t1, t2)
                        nc.vector.tensor_mul(dst, rp, rsb)
                    else:
                        # rs_k = (ssq + D*eps)^-1/2, folded into the exp scale
                        lt = small.tile([1, CW], f32, tag="lt")
                        nc.scalar.activation(lt, ssq, AF.Ln, bias=eps_row)
                        rskr = small.tile([1, CW], f32, tag="rskr")
                        nc.scalar.activation(rskr, lt, AF.Exp, scale=-0.5)
                        nc.sync.dma_start(rsk_dram[:, csl], rskr)
                        nc.sync.dma_start(
                            rsk_cols[c],
                            rsk_dram[:, csl].rearrange("o (t p) -> o p t", p=128)[0],
                        )
                        nc.vector.tensor_add(dst, t1, t2)

                # ---------- causal attention for one (head, query-chunk) ----------
                def att_block(j, ic):
                    isl = slice(ic * CW, (ic + 1) * CW)
                    o_ps = pc.tile([128, CW], f32, tag="big", name="ops")
                    l_ps = psm.tile([1, CW], f32, tag="ssq", name="lps")
                    njt = 4 * ic + 4  # causal: j-tiles 0 .. 4*ic+3

                    def qk_exp(jt):
                        t_ = jt - 4 * ic
                        lo = t_ * 128 if t_ > 0 else 0
                        jsl = slice(jt * 128, (jt + 1) * 128)
                        st = pc.tile([128, CW], f32, tag="big", name="st")
                        nc.tensor.matmul(
                            st[:, lo:],
                            kt_sb[:, jsl],
                            qt_sb[:, j, ic * CW + lo : (ic + 1) * CW],
                            start=True, stop=True,
                        )
                        p = work.tile([128, CW], bf16, tag="p", bufs=8, name="p")
                        nc.scalar.activation(
                            p[:, lo:], st[:, lo:], AF.Exp,
                            scale=rsk_cols[jt // 4][:, jt % 4 : jt % 4 + 1],
                        )
                        if t_ >= 0:  # diagonal block: causal mask
                            nc.vector.tensor_mul(
                                p[:, lo:], p[:, lo:], msk_sb[:, t_, lo:]
                            )
                        return (p, lo)

                    def av(jt, plo):
                        p, lo = plo
                        mm = dict(start=(jt == 0), stop=(jt == njt - 1))
                        nc.tensor.matmul(o_ps[:, lo:], v_sb[:, jt, :], p[:, lo:], **mm)
                        nc.tensor.matmul(l_ps[:, lo:], ones_col_bf, p[:, lo:], **mm)

                    pipe = [qk_exp(0)]
                    if njt > 1:
                        pipe.append(qk_exp(1))
                    for jt in range(2, njt):
                        cur = qk_exp(jt)
                        av(jt - 2, pipe.pop(0))
                        pipe.append(cur)
                    for k_, p_ in enumerate(pipe):
                        av(njt - len(pipe) + k_, p_)
                    # linv = 1/l via Ln+Exp on scalar (avoids slow DVE recip)
                    llog = small.tile([1, CW], f32, tag="llog")
                    nc.scalar.activation(llog, l_ps, AF.Ln)
                    linv_bf = small.tile([1, CW], bf16, tag="linv_bf")
                    nc.scalar.activation(linv_bf, llog, AF.Exp, scale=-1.0)
                    li_dr = dram.tile([1, CW], bf16, tag="li_dr", bufs=3, name="li_dr")
                    nc.sync.dma_start(li_dr, linv_bf)
                    lb = work.tile([128, CW], bf16, tag="lb", bufs=3)
                    nc.sync.dma_start(lb, li_dr[:, :].to_broadcast([128, CW]))
                    nc.vector.tensor_mul(ot_sb[:, j, isl], o_ps, lb)
                    nc.sync.dma_start(
                        a2a_in[j][:, :].rearrange("(r p) s -> p r s", p=128)[
                            :, 2 * ic : 2 * ic + 2, :
                        ],
                        ot_sb[:, j, ic * CW : (ic + 1) * CW].rearrange(
                            "p (r s) -> p r s", r=2
                        ),
                    )

                def a2a(j):
                    nc.gpsimd.collective_compute(
                        "AllToAll",
                        mybir.AluOpType.bypass,
                        replica_groups=[list(range(NC))],
                        ins=[a2a_in[j][:, :].opt()],
                        outs=[a2a_out[j][:, :].opt()],
                    )
                    # readback: a2a_out[j] block g (g = src core) = global head
                    # 2g+j -> att_sb tile index ht = 2g + j
                    nc.sync.dma_start(
                        att_sb[:, :, :, :].rearrange(
                            "p (g j) u s -> j p g u s", j=HL
                        )[j],
                        a2a_out[j][:, :].rearrange("(g p) (u s) -> p g u s", p=128, u=2),
                    )

                for c in range(CH):
                    csl = slice(c * CW, (c + 1) * CW)
                    # xt tiles for this chunk: one DMA + one sem for all 16 MMs
                    # (chunk 0 loads per-ht so the very first matmul only waits
                    # on 128KB, not the whole 2MB)
                    xtt = xtp.tile([128, KT, CW], bf16, tag="xtt", bufs=2)
                    if c == 0:
                        for ht in range(KT):
                            nc.sync.dma_start(
                                xtt[:, ht, :], xt_d[ht * 128 : (ht + 1) * 128, csl]
                            )
                    else:
                        nc.sync.dma_start(
                            xtt, xt_d[:, csl].rearrange("(t p) s -> p t s", p=128)
                        )
                    # pass 1: the two local q heads
                    q_ps = [
                        pa.tile([128, CW], f32, tag="acc", name=f"qps{jj}")
                        for jj in range(HL)
                    ]
                    for ht in range(KT):
                        mm = dict(start=(ht == 0), stop=(ht == KT - 1))
                        for jj in range(HL):
                            nc.tensor.matmul(
                                q_ps[jj], wq_sb[:, ht, jj * D : (jj + 1) * D],
                                xtt[:, ht, :], **mm,
                            )
                    for jj in range(HL):
                        norm_rope(q_ps[jj], c, cq_sb, sq_sb, qt_sb[:, jj, csl], True)
                    # pass 2: k and v for the local kv head
                    k_ps = pa.tile([128, CW], f32, tag="acc", name="kps")
                    v_ps = pa.tile([128, CW], f32, tag="acc", name="vps")
                    for ht in range(KT):
                        mm = dict(start=(ht == 0), stop=(ht == KT - 1))
                        nc.tensor.matmul(k_ps, wk_sb[:, ht, :], xtt[:, ht, :], **mm)
                        nc.tensor.matmul(v_ps, wv_sb[:, ht, :], xtt[:, ht, :], **mm)
                    nc.vector.tensor_copy(vt_sb[:, csl], v_ps)
                    for jt in range(4 * c, 4 * c + 4):
                        nc.sync.dma_start_transpose(
                            v_sb[:, jt, :], vt_sb[:, jt * 128 : (jt + 1) * 128]
                        )
                    norm_rope(k_ps, c, ck_sb, sk_sb, kt_sb[:, csl], False)

                    # head-0 attention, one chunk behind: its norm/rope/rsk
                    # dependencies are long since ready, so PE never stalls
                    # (stall => HAM re-throttles the PE clock to 1.2 GHz)
                    if c > 0:
                        att_block(0, c - 1)

                att_block(0, CH - 1)
                # first collective goes out while head-1 attention runs
                a2a(0)
                # prefetch Wo during head-1 attention (issue point sets the
                # DMA priority: early enough to hide, late enough not to
                # starve phase-A xt streaming)
                wo_ts = []

                def wo_prefetch(g):
                    osl2 = slice(g * 2 * CW, (g + 1) * 2 * CW)
                    for ht in [*range(0, KT, 2), *range(1, KT, 2)]:
                        wo_t = wop.tile([128, 2 * CW], bf16, tag="wot", bufs=12)
                        nc.sync.dma_start(wo_t, wo_d[ht * 128 : (ht + 1) * 128, osl2])
                        wo_ts.append(wo_t)

                wo_prefetch(0)
                for ic in range(CH):
                    att_block(1, ic)
                    if ic == 1:
                        wo_prefetch(1)
                a2a(1)

                # ---------- output projection + final RMSNorm ----------
                for g in range(2):
                    osl2 = slice(g * 2 * CW, (g + 1) * 2 * CW)
                    y_ps = [
                        (pa if i < 3 else pc).tile(
                            [128, CW], f32, tag="acc" if i < 3 else "big",
                            name=f"yps{i}",
                        )
                        for i in range(4)
                    ]
                    for hi, ht in enumerate([*range(0, KT, 2), *range(1, KT, 2)]):
                        wo_t = wo_ts[g * KT + hi]
                        mm = dict(start=(hi == 0), stop=(hi == KT - 1))
                        for st in range(2):
                            for oh in range(2):
                                nc.tensor.matmul(
                                    y_ps[st * 2 + oh],
                                    att_sb[:, ht, st, :],
                                    wo_t[:, oh * CW : (oh + 1) * CW],
                                    **mm,
                                )
                    for st in range(2):
                        for oh in range(2):
                            oc = g * 2 + oh
                            ysl = y_sb[:, st, oc * CW : (oc + 1) * CW]
                            nc.vector.tensor_copy(ysl, y_ps[st * 2 + oh])
                            ysq = work.tile([128, CW], f32, tag="sq2f")
                            nc.vector.tensor_mul(ysq, ysl, ysl)
                            nc.vector.reduce_sum(
                                pt_sb[:, st, oc : oc + 1], ysq, axis=mybir.AxisListType.X
                            )
                for st in range(2):
                    tot = small.tile([128, 1], f32, tag="tot")
                    nc.vector.reduce_sum(tot, pt_sb[:, st, :], axis=mybir.AxisListType.X)
                    yl = small.tile([128, 1], f32, tag="yl")
                    nc.scalar.activation(yl, tot, AF.Ln, bias=eps_col, scale=1.0 / H)
                    rsy = small.tile([128, 1], f32, tag="rsy")
                    nc.scalar.activation(rsy, yl, AF.Exp, scale=-0.5)
                    for half in range(2):
                        hsl = slice(half * 1024, (half + 1) * 1024)
                        o1 = work.tile([128, 1024], f32, tag="o1", bufs=2)
                        nc.vector.tensor_mul(o1, y_sb[:, st, hsl], ls_sb[:, hsl])
                        nc.vector.tensor_scalar_mul(o1, o1, rsy)
                        nc.sync.dma_start(out_d[st * 128 : (st + 1) * 128, hsl], o1)

    nc.compile()
    return nc


def _get_nc(reps: int = 1):
    key = f"nc{reps}"
    if key not in _cache:
        _cache[key] = _build_nc(reps)
    return _cache[key]


def _prep_in_maps(
    hidden_states, cos, sin, Wq, Wk, Wv, Wo, q_norm_scale, k_norm_scale,
    last_norm_scale, attention_mask,
):
    xt = np.ascontiguousarray(np.asarray(hidden_states, np.float32)[0].T).astype(BF16)
    wo = np.ascontiguousarray(np.asarray(Wo, np.float32)).astype(BF16)
    cosr = np.asarray(cos, np.float32)[:, 0, :]  # [S, D]
    sinr = np.asarray(sin, np.float32)[:, 0, :]

    def rope_tables(scale):
        sc = np.asarray(scale, np.float32)
        c_eff = np.ascontiguousarray(cosr.T * sc[:, None]).astype(BF16)  # [D, S]
        rsc = np.concatenate([sc[64:], sc[:64]])  # scale[(d+64)%128]
        s_eff = sinr.T * rsc[:, None]
        return c_eff, np.ascontiguousarray(s_eff).astype(BF16)

    cq, sq = rope_tables(q_norm_scale)
    ck, sk = rope_tables(k_norm_scale)

    msk = np.zeros((D, 4, CW), np.float32)
    jj = np.arange(128)[:, None]
    ii = np.arange(CW)[None, :]
    for t in range(4):
        msk[:, t, :] = (ii >= jj + t * 128).astype(np.float32)
    msk = msk.astype(BF16)
    # R^T for rotate-half-as-matmul: out = R @ q, R[d, d+64] = -1 (d<64),
    # R[d, d-64] = +1 (d>=64); lhsT = R^T
    rotm = np.zeros((D, D), np.float32)
    rotm[np.arange(64) + 64, np.arange(64)] = -1.0
    rotm[np.arange(64), np.arange(64) + 64] = 1.0
    rotm = rotm.astype(BF16)
    ls = np.ascontiguousarray(np.asarray(last_norm_scale, np.float32).reshape(1, H))

    def pack_w(w):
        # [H, C] -> [128, KT, C] with w[t*128+p, c] at [p, t, c]
        return np.ascontiguousarray(
            np.asarray(w, np.float32).reshape(KT, 128, -1).transpose(1, 0, 2)
        ).astype(BF16)

    Wq = np.asarray(Wq, np.float32)
    Wk = np.asarray(Wk, np.float32)
    Wv = np.asarray(Wv, np.float32)
    in_maps = []
    for i in range(NC):
        kv = i // 2
        in_maps.append(
            {
                "xt": xt,
                "wq": pack_w(Wq[:, i * HL * D : (i + 1) * HL * D]),
                "wk": pack_w(Wk[:, kv * D : (kv + 1) * D]),
                "wv": pack_w(Wv[:, kv * D : (kv + 1) * D]),
                "wo": wo,
                "cq": cq,
                "sq": sq,
                "ck": ck,
                "sk": sk,
                "msk": msk,
                "rot": rotm,
                "ls": ls,
            }
        )
    return in_maps


last_results = None


def kernel(**inputs) -> np.ndarray:
    global last_results
    from concourse import bass_utils

    nc = _get_nc()
    in_maps = _prep_in_maps(**inputs)
    res = bass_utils.run_bass_kernel_spmd(nc, in_maps, core_ids=list(range(NC)))
    last_results = res
    parts = [np.asarray(res.results[i]["out"], np.float32) for i in range(NC)]
    return np.concatenate(parts, axis=0)[None, :, :]


# revision 24
# speedup vs baseline: 1.0157x; 1.0157x over previous
"""Distributed GQA attention block (RMSNorm-QK + RoPE + causal attention + Wo
+ final RMSNorm) for one TRN2 chip (8 NeuronCores).

Sharding: tensor-parallel over heads. Core i computes q-heads {2i, 2i+1} and
kv-head i//2. Everything on-device is computed transposed ([dim, seq]) so the
hidden/contraction axis lands on SBUF partitions with zero on-device
transposes of X. An AllToAll redistributes the attention output from
head-sharded to sequence-sharded; each core then runs the output projection
and final RMSNorm for its own 256 tokens.

v3 structure: projection/norm/rope and head-0 attention are software-
pipelined per 512-token chunk; head-1 attention runs after the first
AllToAll is issued so the collective hides under it; the output projection
starts on the even head-blocks as soon as the first AllToAll lands. All
rsqrt/reciprocal chains run as Ln+Exp on the scalar engine (single
activation-table set, no serialized [1,N] DVE reciprocals).

Numerics: bf16 matmuls with f32 PSUM accumulation; softmax is computed
without max-subtraction (logits are O(1) here: RMS-normed q/k and 1/sqrt(D)
scaling), with the causal mask applied multiplicatively after exp.
"""

import sys

sys.path.insert(0, "/opt/trn_rl_repo")

import numpy as np
import ml_dtypes

BF16 = ml_dtypes.bfloat16

S = 2048  # sequence length
H = 2048  # hidden
D = 128  # head dim
NH = 16  # query heads
NKV = 4  # kv heads
NC = 8  # cores
HL = NH // NC  # q heads per core = 2
SC = S // NC  # seq per core (output shard) = 256
CH = 4  # seq chunks
CW = 512  # chunk width
KT = H // 128  # contraction tiles = 16
EPS = 1e-6
SQRT_D = float(np.sqrt(D))

_cache: dict = {}


def _patch_act_tables():
    """Force all activations into the natural_log_exp_and_others table set.

    bacc's insert_act_table_loads picks the FIRST act_info.json set
    containing each function, so alternating Ln/Exp thrashes between
    `natural_log` and `exp_and_others` (~1.3us per ACT_TABLE_LOAD, dozens
    per kernel). Blanking every other set (order/indices preserved, so the
    emitted act_func_set_id still matches act_info.json) makes Ln, Exp and
    Copy all resolve to the one combined set -> a single table load.
    """
    import concourse.bacc as bacc_mod
    import concourse.hw_specs as hw

    orig = hw.get_activation_tables
    if getattr(bacc_mod.get_activation_tables, "_combined_set_only", False):
        return
    KEEP = "natural_log_exp_and_others"

    def patched(arch):
        t = dict(orig(arch))
        return {k: (v if k == KEEP else set()) for k, v in t.items()}

    patched._combined_set_only = True
    bacc_mod.get_activation_tables = patched


def _build_nc(reps: int = 1):
    import concourse.bass as bass
    import concourse.tile as tile
    from concourse import bacc, mybir

    _patch_act_tables()

    f32 = mybir.dt.float32
    bf16 = mybir.dt.bfloat16
    AF = mybir.ActivationFunctionType

    nc = bacc.Bacc("TRN2", target_bir_lowering=False, debug=False, num_devices=NC)

    # ---- kernel I/O (per-core shards; replicated where noted) ----
    xt_d = nc.dram_tensor("xt", [H, S], bf16, kind="ExternalInput").ap()  # X^T
    wq_d = nc.dram_tensor("wq", [128, KT, HL * D], bf16, kind="ExternalInput").ap()
    wk_d = nc.dram_tensor("wk", [128, KT, D], bf16, kind="ExternalInput").ap()
    wv_d = nc.dram_tensor("wv", [128, KT, D], bf16, kind="ExternalInput").ap()
    wo_d = nc.dram_tensor("wo", [H, H], bf16, kind="ExternalInput").ap()
    cq_d = nc.dram_tensor("cq", [D, S], bf16, kind="ExternalInput").ap()
    sq_d = nc.dram_tensor("sq", [D, S], bf16, kind="ExternalInput").ap()
    ck_d = nc.dram_tensor("ck", [D, S], bf16, kind="ExternalInput").ap()
    sk_d = nc.dram_tensor("sk", [D, S], bf16, kind="ExternalInput").ap()
    msk_d = nc.dram_tensor("msk", [D, 4, CW], bf16, kind="ExternalInput").ap()
    rot_d = nc.dram_tensor("rot", [D, D], bf16, kind="ExternalInput").ap()
    ls_d = nc.dram_tensor("ls", [1, H], f32, kind="ExternalInput").ap()
    out_d = nc.dram_tensor("out", [SC, H], f32, kind="ExternalOutput").ap()

    with tile.TileContext(nc) as tc:
        with (
            tc.tile_pool(name="singles", bufs=1) as singles,
            tc.tile_pool(name="work", bufs=3) as work,
            tc.tile_pool(name="xtp", bufs=18) as xtp,
            tc.tile_pool(name="wop", bufs=8) as wop,
            tc.tile_pool(name="small", bufs=2) as small,
            tc.tile_pool(name="pa", bufs=3, space="PSUM") as pa,
            tc.tile_pool(name="pc", bufs=4, space="PSUM") as pc,
            tc.tile_pool(name="psm", bufs=1, space="PSUM") as psm,
            tc.tile_pool(name="dram", bufs=1, space="DRAM") as dram,
        ):
            # ---------- resident SBUF tensors ----------
            # q/k/v weights first: the first projection matmuls need them
            wq_sb = singles.tile([128, KT, HL * D], bf16)
            wk_sb = singles.tile([128, KT, D], bf16)
            wv_sb = singles.tile([128, KT, D], bf16)
            for g in range(4):
                gs = slice(4 * g, 4 * g + 4)
                nc.sync.dma_start(wq_sb[:, gs, :], wq_d[:, gs, :])
                nc.sync.dma_start(wk_sb[:, gs, :], wk_d[:, gs, :])
                nc.sync.dma_start(wv_sb[:, gs, :], wv_d[:, gs, :])
            rot_sb = singles.tile([128, 128], bf16)  # R^T: rotate-half as matmul
            nc.sync.dma_start(rot_sb, rot_d)
            cq_sb = singles.tile([128, S], bf16)
            sq_sb = singles.tile([128, S], bf16)
            ck_sb = singles.tile([128, S], bf16)
            sk_sb = singles.tile([128, S], bf16)
            for hh in range(2):
                hs = slice(hh * 1024, (hh + 1) * 1024)
                nc.sync.dma_start(cq_sb[:, hs], cq_d[:, hs])
                nc.sync.dma_start(sq_sb[:, hs], sq_d[:, hs])
                nc.sync.dma_start(ck_sb[:, hs], ck_d[:, hs])
                nc.sync.dma_start(sk_sb[:, hs], sk_d[:, hs])
            msk_sb = singles.tile([128, 4, CW], bf16)
            nc.sync.dma_start(msk_sb, msk_d)
            ls_sb = singles.tile([128, H], f32)  # last_norm_scale bcast over parts
            nc.sync.dma_start(ls_sb, ls_d.to_broadcast([128, H]))

            ones_col_bf = singles.tile([128, 1], bf16)  # lhsT for partition sums
            nc.vector.memset(ones_col_bf, 1.0)
            eps_row = singles.tile([1, 1], f32)  # D*eps for k sumsq
            nc.vector.memset(eps_row, D * EPS)
            eps_one = singles.tile([1, 1], f32)  # eps for q mean-sumsq
            nc.vector.memset(eps_one, EPS)
            eps_col = singles.tile([128, 1], f32)  # eps for final norm
            nc.vector.memset(eps_col, EPS)

            qt_sb = singles.tile([128, HL, S], bf16)  # roped Q^T per local head
            kt_sb = singles.tile([128, S], bf16)  # roped K^T (rs_k deferred)
            vt_sb = singles.tile([128, S], bf16)  # V^T (pre-transpose)
            v_sb = singles.tile([128, KT, D], bf16)  # V tiles [j-part, jt, d]
            rsk_cols = [
                singles.tile([128, 4], f32, name=f"rskc{c}") for c in range(CH)
            ]  # partition layout, per source chunk
            ot_sb = singles.tile([128, HL, S], bf16)  # normalized attn out^T
            att_sb = singles.tile([128, KT, HL, 128], bf16)  # att^T post-A2A
            y_sb = singles.tile([128, 2, H], bf16)  # Y rows (2 x 128 tokens)
            pt_sb = singles.tile([128, 2, 4], f32)  # sumsq partials per col group
            a2a_in = [
                dram.tile([S // 2, SC], bf16, name=f"a2ai{j}") for j in range(HL)
            ]
            a2a_out = [
                dram.tile([S // 2, SC], bf16, name=f"a2ao{j}") for j in range(HL)
            ]
            rsk_dram = dram.tile([1, S], f32)

            for _rep in range(reps):
                # ---------- per-chunk: QKV projection, RMSNorm, RoPE ----------
                def norm_rope(src_ps, c, cos_sb, sin_sb, dst, is_q):
                    """src_ps: [128,CW] f32 PSUM (pre-norm, transposed)."""
                    csl = slice(c * CW, (c + 1) * CW)
                    raw = work.tile([128, CW], bf16, tag="raw", bufs=3)
                    nc.vector.tensor_copy(raw, src_ps)  # frees the PSUM bank
                    sq2 = work.tile([128, CW], bf16, tag="sq2")
                    nc.vector.tensor_mul(sq2, raw, raw)
                    ssq = psm.tile([1, CW], f32, tag="ssq")
                    nc.tensor.matmul(ssq, ones_col_bf, sq2, start=True, stop=True)
                    # rope on raw runs in parallel with the rs chain
                    rot_ps = pc.tile([128, CW], f32, tag="big", name="rotps")
                    nc.tensor.matmul(rot_ps, rot_sb, raw, start=True, stop=True)
                    t1 = work.tile([128, CW], bf16, tag="t1")
                    nc.vector.tensor_mul(t1, raw, cos_sb[:, csl])
                    t2 = work.tile([128, CW], bf16, tag="t2")
                    nc.vector.tensor_mul(t2, rot_ps, sin_sb[:, csl])
                    if is_q:
                        # rs = (ssq/D + eps)^-1/2 via Ln+Exp (one act-table set)
                        lt = small.tile([1, CW], f32, tag="lt")
                        nc.scalar.activation(lt, ssq, AF.Ln, bias=eps_one, scale=1.0 / D)
                        rs_bf = small.tile([1, CW], bf16, tag="rs_bf")
                        nc.scalar.activation(rs_bf, lt, AF.Exp, scale=-0.5)
                        rs_dr = dram.tile([1, CW], bf16, tag="rs_dr", bufs=3, name="rs_dr")
                        nc.sync.dma_start(rs_dr, rs_bf)
                        rsb = work.tile([128, CW], bf16, tag="rsb", bufs=3)
                        nc.sync.dma_start(rsb, rs_dr[:, :].to_broadcast([128, CW]))
                        rp = work.tile([128, CW], bf16, tag="rp")
                        nc.vector.tensor_add(rp, t1, t2)
                        nc.vector.tensor_mul(dst, rp, rsb)
                    else:
                        # rs_k = (ssq + D*eps)^-1/2, folded into the exp scale
                        lt = small.tile([1, CW], f32, tag="lt")
                        nc.scalar.activation(lt, ssq, AF.Ln, bias=eps_row)
                        rskr = small.tile([1, CW], f32, tag="rskr")
                        nc.scalar.activation(rskr, lt, AF.Exp, scale=-0.5)
                        nc.sync.dma_start(rsk_dram[:, csl], rskr)
                        nc.sync.dma_start(
                            rsk_cols[c],
                            rsk_dram[:, csl].rearrange("o (t p) -> o p t", p=128)[0],
                        )
                        nc.vector.tensor_add(dst, t1, t2)

                # ---------- causal attention for one (head, query-chunk) ----------
                def att_block(j, ic):
                    isl = slice(ic * CW, (ic + 1) * CW)
                    o_ps = pc.tile([128, CW], f32, tag="big", name="ops")
                    l_ps = psm.tile([1, CW], f32, tag="ssq", name="lps")
                    njt = 4 * ic + 4  # causal: j-tiles 0 .. 4*ic+3

                    def qk_exp(jt):
                        t_ = jt - 4 * ic
                        lo = t_ * 128 if t_ > 0 else 0
                        jsl = slice(jt * 128, (jt + 1) * 128)
                        st = pc.tile([128, CW], f32, tag="big", name="st")
                        nc.tensor.matmul(
                            st[:, lo:],
                            kt_sb[:, jsl],
                            qt_sb[:, j, ic * CW + lo : (ic + 1) * CW],
                            start=True, stop=True,
                        )
                        p = work.tile([128, CW], bf16, tag="p", bufs=8, name="p")
                        nc.scalar.activation(
                            p[:, lo:], st[:, lo:], AF.Exp,
                            scale=rsk_cols[jt // 4][:, jt % 4 : jt % 4 + 1],
                        )
                        if t_ >= 0:  # diagonal block: causal mask
                            nc.vector.tensor_mul(
                                p[:, lo:], p[:, lo:], msk_sb[:, t_, lo:]
                            )
                        return (p, lo)

                    def av(jt, plo):
                        p, lo = plo
                        mm = dict(start=(jt == 0), stop=(jt == njt - 1))
                        nc.tensor.matmul(o_ps[:, lo:], v_sb[:, jt, :], p[:, lo:], **mm)
                        nc.tensor.matmul(l_ps[:, lo:], ones_col_bf, p[:, lo:], **mm)

                    pipe = [qk_exp(0)]
                    if njt > 1:
                        pipe.append(qk_exp(1))
                    for jt in range(2, njt):
                        cur = qk_exp(jt)
                        av(jt - 2, pipe.pop(0))
                        pipe.append(cur)
                    for k_, p_ in enumerate(pipe):
                        av(njt - len(pipe) + k_, p_)
                    # linv = 1/l via Ln+Exp on scalar (avoids slow DVE recip)
                    llog = small.tile([1, CW], f32, tag="llog")
                    nc.scalar.activation(llog, l_ps, AF.Ln)
                    linv_bf = small.tile([1, CW], bf16, tag="linv_bf")
                    nc.scalar.activation(linv_bf, llog, AF.Exp, scale=-1.0)
                    li_dr = dram.tile([1, CW], bf16, tag="li_dr", bufs=3, name="li_dr")
                    nc.sync.dma_start(li_dr, linv_bf)
                    lb = work.tile([128, CW], bf16, tag="lb", bufs=3)
                    nc.sync.dma_start(lb, li_dr[:, :].to_broadcast([128, CW]))
                    nc.vector.tensor_mul(ot_sb[:, j, isl], o_ps, lb)
                    nc.sync.dma_start(
                        a2a_in[j][:, :].rearrange("(r p) s -> p r s", p=128)[
                            :, 2 * ic : 2 * ic + 2, :
                        ],
                        ot_sb[:, j, ic * CW : (ic + 1) * CW].rearrange(
                            "p (r s) -> p r s", r=2
                        ),
                    )

                def a2a(j):
                    nc.gpsimd.collective_compute(
                        "AllToAll",
                        mybir.AluOpType.bypass,
                        replica_groups=[list(range(NC))],
                        ins=[a2a_in[j][:, :].opt()],
                        outs=[a2a_out[j][:, :].opt()],
                    )
                    # readback: a2a_out[j] block g (g = src core) = global head
                    # 2g+j -> att_sb tile index ht = 2g + j
                    nc.sync.dma_start(
                        att_sb[:, :, :, :].rearrange(
                            "p (g j) u s -> j p g u s", j=HL
                        )[j],
                        a2a_out[j][:, :].rearrange("(g p) (u s) -> p g u s", p=128, u=2),
                    )

                for c in range(CH):
                    csl = slice(c * CW, (c + 1) * CW)
                    # xt tiles for this chunk: one DMA + one sem for all 16 MMs
                    # (chunk 0 loads per-ht so the very first matmul only waits
                    # on 128KB, not the whole 2MB)
                    xtt = xtp.tile([128, KT, CW], bf16, tag="xtt", bufs=2)
                    if c == 0:
                        for ht in range(KT):
                            nc.sync.dma_start(
                                xtt[:, ht, :], xt_d[ht * 128 : (ht + 1) * 128, csl]
                            )
                    else:
                        nc.sync.dma_start(
                            xtt, xt_d[:, csl].rearrange("(t p) s -> p t s", p=128)
                        )
                    # pass 1: the two local q heads
                    q_ps = [
                        pa.tile([128, CW], f32, tag="acc", name=f"qps{jj}")
                        for jj in range(HL)
                    ]
                    for ht in range(KT):
                        mm = dict(start=(ht == 0), stop=(ht == KT - 1))
                        for jj in range(HL):
                            nc.tensor.matmul(
                                q_ps[jj], wq_sb[:, ht, jj * D : (jj + 1) * D],
                                xtt[:, ht, :], **mm,
                            )
                    for jj in range(HL):
                        norm_rope(q_ps[jj], c, cq_sb, sq_sb, qt_sb[:, jj, csl], True)
                    # pass 2: k and v for the local kv head
                    k_ps = pa.tile([128, CW], f32, tag="acc", name="kps")
                    v_ps = pa.tile([128, CW], f32, tag="acc", name="vps")
                    for ht in range(KT):
                        mm = dict(start=(ht == 0), stop=(ht == KT - 1))
                        nc.tensor.matmul(k_ps, wk_sb[:, ht, :], xtt[:, ht, :], **mm)
                        nc.tensor.matmul(v_ps, wv_sb[:, ht, :], xtt[:, ht, :], **mm)
                    nc.vector.tensor_copy(vt_sb[:, csl], v_ps)
                    for jt in range(4 * c, 4 * c + 4):
                        nc.sync.dma_start_transpose(
                            v_sb[:, jt, :], vt_sb[:, jt * 128 : (jt + 1) * 128]
                        )
                    norm_rope(k_ps, c, ck_sb, sk_sb, kt_sb[:, csl], False)

                    # head-0 attention, one chunk behind: its norm/rope/rsk
                    # dependencies are long since ready, so PE never stalls
                    # (stall => HAM re-throttles the PE clock to 1.2 GHz)
                    if c > 0:
                        att_block(0, c - 1)

                att_block(0, CH - 1)
                # first collective goes out while head-1 attention runs
                a2a(0)
                # prefetch Wo during head-1 attention (issue point sets the
                # DMA priority: early enough to hide, late enough not to
                # starve phase-A xt streaming)
                wo_ts = []

                def wo_prefetch(g):
                    osl2 = slice(g * 2 * CW, (g + 1) * 2 * CW)
                    for ht in [*range(0, KT, 2), *range(1, KT, 2)]:
                        wo_t = wop.tile([128, 2 * CW], bf16, tag="wot", bufs=12)
                        nc.sync.dma_start(wo_t, wo_d[ht * 128 : (ht + 1) * 128, osl2])
                        wo_ts.append(wo_t)

                wo_prefetch(0)
                for ic in range(CH):
                    att_block(1, ic)
                    if ic == 1:
                        wo_prefetch(1)
                a2a(1)

                # ---------- output projection + final RMSNorm ----------
                for g in range(2):
                    osl2 = slice(g * 2 * CW, (g + 1) * 2 * CW)
                    y_ps = [
                        (pa if i < 3 else pc).tile(
                            [128, CW], f32, tag="acc" if i < 3 else "big",
                            name=f"yps{i}",
                        )
                        for i in range(4)
                    ]
                    for hi, ht in enumerate([*range(0, KT, 2), *range(1, KT, 2)]):
                        wo_t = wo_ts[g * KT + hi]
                        mm = dict(start=(hi == 0), stop=(hi == KT - 1))
                        for st in range(2):
                            for oh in range(2):
                                nc.tensor.matmul(
                                    y_ps[st * 2 + oh],
                                    att_sb[:, ht, st, :],
                                    wo_t[:, oh * CW : (oh + 1) * CW],
                                    **mm,
                                )
                    for st in range(2):
                        for oh in range(2):
                            oc = g * 2 + oh
                            ysl = y_sb[:, st, oc * CW : (oc + 1) * CW]
                            nc.vector.tensor_copy(ysl, y_ps[st * 2 + oh])
                            ysq = work.tile([128, CW], f32, tag="sq2f")
                            nc.vector.tensor_mul(ysq, ysl, ysl)
                            nc.vector.reduce_sum(
                                pt_sb[:, st, oc : oc + 1], ysq, axis=mybir.AxisListType.X
                            )
                for st in range(2):
                    tot = small.tile([128, 1], f32, tag="tot")
                    nc.vector.reduce_sum(tot, pt_sb[:, st, :], axis=mybir.AxisListType.X)
                    yl = small.tile([128, 1], f32, tag="yl")
                    nc.scalar.activation(yl, tot, AF.Ln, bias=eps_col, scale=1.0 / H)
                    rsy = small.tile([128, 1], f32, tag="rsy")
                    nc.scalar.activation(rsy, yl, AF.Exp, scale=-0.5)
                    for half in range(2):
                        hsl = slice(half * 1024, (half + 1) * 1024)
                        o1 = work.tile([128, 1024], f32, tag="o1", bufs=2)
                        nc.vector.tensor_mul(o1, y_sb[:, st, hsl], ls_sb[:, hsl])
                        nc.vector.tensor_scalar_mul(o1, o1, rsy)
                        nc.sync.dma_start(out_d[st * 128 : (st + 1) * 128, hsl], o1)

    nc.compile()
    return nc


def _get_nc(reps: int = 1):
    key = f"nc{reps}"
    if key not in _cache:
        _cache[key] = _build_nc(reps)
    return _cache[key]


def _prep_in_maps(
    hidden_states, cos, sin, Wq, Wk, Wv, Wo, q_norm_scale, k_norm_scale,
    last_norm_scale, attention_mask,
):
    xt = np.ascontiguousarray(np.asarray(hidden_states, np.float32)[0].T).astype(BF16)
    wo = np.ascontiguousarray(np.asarray(Wo, np.float32)).astype(BF16)
    cosr = np.asarray(cos, np.float32)[:, 0, :]  # [S, D]
    sinr = np.asarray(sin, np.float32)[:, 0, :]

    def rope_tables(scale):
        sc = np.asarray(scale, np.float32)
        c_eff = np.ascontiguousarray(cosr.T * sc[:, None]).astype(BF16)  # [D, S]
        rsc = np.concatenate([sc[64:], sc[:64]])  # scale[(d+64)%128]
        s_eff = sinr.T * rsc[:, None]
        return c_eff, np.ascontiguousarray(s_eff).astype(BF16)

    cq, sq = rope_tables(q_norm_scale)
    ck, sk = rope_tables(k_norm_scale)

    msk = np.zeros((D, 4, CW), np.float32)
    jj = np.arange(128)[:, None]
    ii = np.arange(CW)[None, :]
    for t in range(4):
        msk[:, t, :] = (ii >= jj + t * 128).astype(np.float32)
    msk = msk.astype(BF16)
    # R^T for rotate-half-as-matmul: out = R @ q, R[d, d+64] = -1 (d<64),
    # R[d, d-64] = +1 (d>=64); lhsT = R^T
    rotm = np.zeros((D, D), np.float32)
    rotm[np.arange(64) + 64, np.arange(64)] = -1.0
    rotm[np.arange(64), np.arange(64) + 64] = 1.0
    rotm = rotm.astype(BF16)
    ls = np.ascontiguousarray(np.asarray(last_norm_scale, np.float32).reshape(1, H))

    def pack_w(w):
        # [H, C] -> [128, KT, C] with w[t*128+p, c] at [p, t, c]
        return np.ascontiguousarray(
            np.asarray(w, np.float32).reshape(KT, 128, -1).transpose(1, 0, 2)
        ).astype(BF16)

    Wq = np.asarray(Wq, np.float32)
    Wk = np.asarray(Wk, np.float32)
    Wv = np.asarray(Wv, np.float32)
    in_maps = []
    for i in range(NC):
        kv = i // 2
        in_maps.append(
            {
                "xt": xt,
                "wq": pack_w(Wq[:, i * HL * D : (i + 1) * HL * D]),
                "wk": pack_w(Wk[:, kv * D : (kv + 1) * D]),
                "wv": pack_w(Wv[:, kv * D : (kv + 1) * D]),
                "wo": wo,
                "cq": cq,
                "sq": sq,
                "ck": ck,
                "sk": sk,
                "msk": msk,
                "rot": rotm,
                "ls": ls,
            }
        )
    return in_maps


last_results = None


def kernel(**inputs) -> np.ndarray:
    global last_results
    from concourse import bass_utils

    nc = _get_nc()
    in_maps = _prep_in_maps(**inputs)
    res = bass_utils.run_bass_kernel_spmd(nc, in_maps, core_ids=list(range(NC)))
    last_results = res
    parts = [np.asarray(res.results[i]["out"], np.float32) for i in range(NC)]
    return np.concatenate(parts, axis=0)[None, :, :]
